# revision 1
# baseline (speedup 1.0000x reference)
"""Trainium2 Bass kernel for nn_DistanceLayer_52020643889255.

Computes, for K=4 domains x B=4096 anchors x T=32 neighbors:
  nbr_embed[k,b,t,:] = semb[k, topk[k,b,t], :]
  nsim[k,b,t]        = dot(semb[k,b], nbr_embed[k,b,t])
  same[k,b,t]        = slabels[k, topk[k,b,t]] == slabels[k,b]
  pair_mask          = same[:, :, :, None] & ~same[:, :, None, :]
  pos_dist           = where(pair_mask, nsim[..., :, None], 0)
  neg_dist           = where(pair_mask, nsim[..., None, :], 0)
Returns (pos_dist, neg_dist, pair_mask, nbr_embed).

Sharding: batch B split 8 ways (512 anchors/core, all 4 domains on every
core); the gather table (semb rows augmented with the label in column 128,
rows padded to 192 floats for the dma_gather 256B-multiple constraint) is
replicated to every core and gathered from DRAM via the GPSIMD dma_gather
extended instruction, 1024 rows per call (the Q7 idx scratch limit is
between 1024 and 2048).

Per 128-anchor block: 4 gather calls fill G[128p, 32t, 192]; DVE computes
the dot products via broadcast-mul + grouped reduce, the label-equality
masks, and the masked pos/neg grids; HWDGE stores stream the outputs out.
"""
import sys

if "/opt/trn_rl_repo" not in sys.path:
    sys.path.insert(0, "/opt/trn_rl_repo")

import numpy as np

K, B, T, D = 4, 4096, 32, 128
NCORES = 8
BS = B // NCORES          # 512 anchors per core per domain
NBLK = BS // 128          # 4 blocks of 128 anchors
E = 192                   # augmented row: 128 emb + label + pad (768B, %256==0)
NI = 1024                 # idxs per dma_gather call (hw-safe limit)
NQ = T * 128 // NI        # 4 gather calls per block
TQ = T // NQ              # 8 neighbor slots per call

_CACHE = {}


def _build():
    """Build and compile the per-core Bass program (same program for all
    cores; cores differ only in input data)."""
    import concourse.bacc as bacc
    import concourse.mybir as mybir
    import concourse.tile as tile
    from concourse.bass import broadcast_tensor_aps
    from concourse import library_config

    nc = bacc.Bacc("TRN2", target_bir_lowering=False, debug=False)

    tbl = nc.dram_tensor("tbl", [K * B, E], mybir.dt.float32, kind="ExternalInput")
    anch = nc.dram_tensor("anch", [K, NBLK, 128, E], mybir.dt.float32,
                          kind="ExternalInput")
    idxs = nc.dram_tensor("idxs", [128, K * NBLK * NQ * (NI // 16)],
                          mybir.dt.int16, kind="ExternalInput")
    nbr_o = nc.dram_tensor("nbr", [K, BS, T, D], mybir.dt.float32,
                           kind="ExternalOutput")
    pos_o = nc.dram_tensor("pos", [K, BS, T, T], mybir.dt.float32,
                           kind="ExternalOutput")
    neg_o = nc.dram_tensor("neg", [K, BS, T, T], mybir.dt.float32,
                           kind="ExternalOutput")
    msk_o = nc.dram_tensor("msk", [K, BS, T, T], mybir.dt.uint8,
                           kind="ExternalOutput")

    with tile.TileContext(nc) as tc:
        with (
            tc.tile_pool(name="const", bufs=1) as cpool,
            tc.tile_pool(name="work", bufs=2) as pool,
        ):
            nc.gpsimd.load_library(library_config.mlp)
            idx_all = cpool.tile([128, K * NBLK * NQ * (NI // 16)], mybir.dt.int16)
            nc.sync.dma_start(idx_all[:], idxs[:])

            for k in range(K):
                tbl_k = tbl[k * B:(k + 1) * B, :]
                for jb in range(NBLK):
                    g = pool.tile([128, T, E], mybir.dt.float32, tag="g")
                    a = pool.tile([128, E], mybir.dt.float32, tag="a")
                    w = pool.tile([128, T, D], mybir.dt.float32, tag="w")
                    s = pool.tile([128, T], mybir.dt.float32, tag="s")
                    sm = pool.tile([128, T], mybir.dt.float32, tag="sm")
                    ns = pool.tile([128, T], mybir.dt.float32, tag="ns")
                    pair = pool.tile([128, T, T], mybir.dt.float32, tag="pair")
                    pos = pool.tile([128, T, T], mybir.dt.float32, tag="pos")
                    neg = pool.tile([128, T, T], mybir.dt.float32, tag="neg")
                    msk = pool.tile([128, T, T], mybir.dt.uint8, tag="msk")

                    # gather: call q covers neighbor slots [q*TQ, (q+1)*TQ);
                    # logical row j = tq*128 + p lands at [p, j//128] of the
                    # out slice, i.e. g[p, q*TQ+tq, :] = tbl_k[topk[p, q*TQ+tq]]
                    for q in range(NQ):
                        col = ((k * NBLK + jb) * NQ + q) * (NI // 16)
                        nc.gpsimd.dma_gather(
                            g[:, q * TQ:(q + 1) * TQ, :], tbl_k,
                            idx_all[:, col:col + NI // 16], NI, NI, E,
                        )
                    nc.sync.dma_start(a[:], anch[k, jb])

                    # nsim: broadcast-mul anchors over neighbor slots, then
                    # grouped reduce over the 128 embedding columns
                    g_b, a_b = broadcast_tensor_aps(g[:, :, 0:D], a[:, None, 0:D])
                    nc.vector.tensor_mul(w[:], g_b, a_b)
                    nc.vector.reduce_sum(s[:], w[:], axis=mybir.AxisListType.X)

                    # same / not-same from the label column
                    gl_b, al_b = broadcast_tensor_aps(g[:, :, D], a[:, None, D])
                    nc.vector.tensor_tensor(sm[:], gl_b, al_b,
                                            mybir.AluOpType.is_equal)
                    nc.vector.tensor_scalar(ns[:], sm[:], -1.0, 1.0,
                                            mybir.AluOpType.mult,
                                            mybir.AluOpType.add)

                    # pair[p, s_, d_] = sm[p, s_] * ns[p, d_]
                    sm_b, ns_b = broadcast_tensor_aps(sm[:, :, None], ns[:, None, :])
                    nc.vector.tensor_mul(pair[:], sm_b, ns_b)
                    p1, s1 = broadcast_tensor_aps(pair[:], s[:, :, None])
                    nc.vector.tensor_mul(pos[:], p1, s1)
                    p2, s2 = broadcast_tensor_aps(pair[:], s[:, None, :])
                    nc.vector.tensor_mul(neg[:], p2, s2)
                    nc.vector.tensor_copy(msk[:], pair[:])

                    b0 = jb * 128
                    nc.sync.dma_start(nbr_o[k, b0:b0 + 128], g[:, :, 0:D])
                    nc.sync.dma_start(pos_o[k, b0:b0 + 128], pos[:])
                    nc.sync.dma_start(neg_o[k, b0:b0 + 128], neg[:])
                    nc.sync.dma_start(msk_o[k, b0:b0 + 128], msk[:])

    nc.compile()
    from concourse.bass_interp import get_hw_module
    nc.m = get_hw_module(nc.m)
    names = dict(tbl=tbl.name, anch=anch.name, idxs=idxs.name,
                 nbr=nbr_o.name, pos=pos_o.name, neg=neg_o.name, msk=msk_o.name)
    return nc, names


def get_program():
    if "prog" not in _CACHE:
        _CACHE["prog"] = _build()
    return _CACHE["prog"]


def make_in_maps(semb, slabels, topk_ngh):
    """Host-side input prep: augmented gather table (replicated), per-core
    anchor rows, and per-core prepacked int16 gather indices."""
    names = get_program()[1]
    semb = np.asarray(semb, dtype=np.float32)
    slabels = np.asarray(slabels)
    topk = np.asarray(topk_ngh)

    tbl_np = np.zeros((K, B, E), np.float32)
    tbl_np[:, :, :D] = semb
    tbl_np[:, :, D] = slabels.astype(np.float32)
    tbl_flat = np.ascontiguousarray(tbl_np.reshape(K * B, E))

    in_maps = []
    for c in range(NCORES):
        lo = c * BS
        anch_np = np.ascontiguousarray(
            tbl_np[:, lo:lo + BS].reshape(K, NBLK, 128, E))
        # idx packing: per (k, jb, q) a [16, NI//16] int16 block in which
        # logical element j (= tq*128 + p) sits at [j % 16, j // 16];
        # replicated across the 8 groups of 16 partitions for the Q7 cores.
        tk = topk[:, lo:lo + BS]  # [K, BS, T]
        blocks = []
        for k in range(K):
            for jb in range(NBLK):
                tkb = tk[k, jb * 128:(jb + 1) * 128]          # [128, T]
                for q in range(NQ):
                    order = tkb[:, q * TQ:(q + 1) * TQ].T.reshape(-1)  # j = tq*128+p
                    blocks.append(order.reshape(NI // 16, 16).T)       # [16, NI//16]
        idx16 = np.concatenate(blocks, axis=1).astype(np.int16)        # [16, ...]
        idx_np = np.ascontiguousarray(np.tile(idx16, (8, 1)))          # [128, ...]
        in_maps.append({names["tbl"]: tbl_flat, names["anch"]: anch_np,
                        names["idxs"]: idx_np})
    return in_maps


def assemble(results):
    """Stitch per-core outputs back to full [K, B, ...] arrays."""
    names = get_program()[1]
    pos = np.concatenate([r[names["pos"]] for r in results], axis=1)
    neg = np.concatenate([r[names["neg"]] for r in results], axis=1)
    msk = np.concatenate([r[names["msk"]] for r in results], axis=1).astype(bool)
    nbr = np.concatenate([r[names["nbr"]] for r in results], axis=1)
    return pos, neg, msk, nbr


def kernel(semb, slabels, topk_ngh):
    from concourse import bass_utils

    nc, _ = get_program()
    in_maps = make_in_maps(semb, slabels, topk_ngh)
    res = bass_utils.run_bass_kernel_spmd(nc, in_maps, core_ids=list(range(NCORES)))
    return assemble(res.results)


# revision 5
# speedup vs baseline: 128.0336x; 128.0336x over previous
"""Trainium2 Bass kernel for nn_DistanceLayer_52020643889255.

Computes, for K=4 domains x B=4096 anchors x T=32 neighbors:
  nbr_embed[k,b,t,:] = semb[k, topk[k,b,t], :]
  nsim[k,b,t]        = dot(semb[k,b], nbr_embed[k,b,t])
  same[k,b,t]        = slabels[k, topk[k,b,t]] == slabels[k,b]
  pair_mask          = same[:, :, :, None] & ~same[:, :, None, :]
  pos_dist           = where(pair_mask, nsim[..., :, None], 0)
  neg_dist           = where(pair_mask, nsim[..., None, :], 0)
Returns (pos_dist, neg_dist, pair_mask, nbr_embed).

Sharding: batch B split 8 ways (512 anchors/core, all 4 domains on every
core). The full semb table is replicated to every core in DRAM and the
neighbor rows (512B each) are gathered straight from it with the GPSIMD
dma_gather extended instruction, 1024 rows per call (the Q7 idx scratch
limit sits between 1024 and 2048 idxs/call). The tiny binary `same` mask
([K,B,T], pure int label/index prep) is precomputed on host and shipped
as an input; all embedding gathers, dot products, and the [T,T] grid
expansion run on device.

Per 128-anchor block: 4 gather calls fill G[128p, 32t, 128d]; DVE computes
nsim via broadcast-mul + grouped reduce, then the masked pos/neg grids as
outer products of premultiplied row/col vectors (pos = (sm*S) x ns,
neg = sm x (ns*S), mask = sm x ns cast to u8); HWDGE streams the outputs.
"""
import sys

if "/opt/trn_rl_repo" not in sys.path:
    sys.path.insert(0, "/opt/trn_rl_repo")

import numpy as np

K, B, T, D = 4, 4096, 32, 128
NCORES = 8
BS = B // NCORES          # 512 anchors per core per domain
NBLK = BS // 128          # 4 blocks of 128 anchors
NI = 1024                 # idxs per dma_gather call (hw-safe limit)
NQ = T * 128 // NI        # 4 gather calls per block
TQ = T // NQ              # 8 neighbor slots per call
IDXCOLS = K * NBLK * NQ * (NI // 16)
SMCOLS = K * NBLK * T

_CACHE = {}


def _build():
    """Build and compile the per-core Bass program (same program for all
    cores; cores differ only in input data)."""
    import concourse.bacc as bacc
    import concourse.mybir as mybir
    import concourse.tile as tile
    from concourse.bass import broadcast_tensor_aps
    from concourse import library_config

    nc = bacc.Bacc("TRN2", target_bir_lowering=False, debug=False,
                   num_swdge_queues=4)

    tbl = nc.dram_tensor("tbl", [K * B, D], mybir.dt.float32, kind="ExternalInput")
    anch = nc.dram_tensor("anch", [K, NBLK, 128, D], mybir.dt.float32,
                          kind="ExternalInput")
    smi = nc.dram_tensor("smi", [128, SMCOLS], mybir.dt.float32,
                         kind="ExternalInput")
    idxs = nc.dram_tensor("idxs", [128, IDXCOLS], mybir.dt.int16,
                          kind="ExternalInput")
    nbr_o = nc.dram_tensor("nbr", [K, BS, T, D], mybir.dt.float32,
                           kind="ExternalOutput")
    pos_o = nc.dram_tensor("pos", [K, BS, T, T], mybir.dt.float32,
                           kind="ExternalOutput")
    neg_o = nc.dram_tensor("neg", [K, BS, T, T], mybir.dt.float32,
                           kind="ExternalOutput")
    msk_o = nc.dram_tensor("msk", [K, BS, T, T], mybir.dt.uint8,
                           kind="ExternalOutput")

    with tile.TileContext(nc) as tc:
        with (
            tc.tile_pool(name="const", bufs=1) as cpool,
            tc.tile_pool(name="work", bufs=3) as pool,
        ):
            nc.gpsimd.load_library(library_config.mlp)
            idx_all = cpool.tile([128, IDXCOLS], mybir.dt.int16)
            sm_all = cpool.tile([128, SMCOLS], mybir.dt.float32)
            nc.sync.dma_start(idx_all[:], idxs[:])
            nc.sync.dma_start(sm_all[:], smi[:])

            for k in range(K):
                tbl_k = tbl[k * B:(k + 1) * B, :]
                for jb in range(NBLK):
                    g = pool.tile([128, T, D], mybir.dt.float32, tag="g")
                    a = pool.tile([128, D], mybir.dt.float32, tag="a")
                    w = pool.tile([128, T, D], mybir.dt.float32, tag="w")
                    s = pool.tile([128, T], mybir.dt.float32, tag="s")
                    ns = pool.tile([128, T], mybir.dt.float32, tag="ns")
                    smS = pool.tile([128, T], mybir.dt.float32, tag="smS")
                    nsS = pool.tile([128, T], mybir.dt.float32, tag="nsS")
                    pos = pool.tile([128, T, T], mybir.dt.float32, tag="pos")
                    neg = pool.tile([128, T, T], mybir.dt.float32, tag="neg")
                    msk = pool.tile([128, T, T], mybir.dt.uint8, tag="msk")

                    # gather: call q covers neighbor slots [q*TQ, (q+1)*TQ);
                    # logical row j = tq*128 + p lands at [p, j//128] of the
                    # out slice, i.e. g[p, q*TQ+tq, :] = tbl_k[topk[p, q*TQ+tq]]
                    # spread the 4 gather calls across the 4 SWDGE queues so
                    # their descriptor packets drain on different SDMA rings
                    # concurrently (measured ~2x vs a single queue)
                    for q in range(NQ):
                        col = ((k * NBLK + jb) * NQ + q) * (NI // 16)
                        nc.gpsimd.dma_gather(
                            g[:, q * TQ:(q + 1) * TQ, :], tbl_k,
                            idx_all[:, col:col + NI // 16], NI, NI, D,
                            queue_num=q,
                        )
                    nc.sync.dma_start(a[:], anch[k, jb])
                    sm = sm_all[:, (k * NBLK + jb) * T:(k * NBLK + jb + 1) * T]

                    # nsim: broadcast-mul anchors over neighbor slots, then
                    # grouped reduce over the 128 embedding columns
                    g_b, a_b = broadcast_tensor_aps(g[:], a[:, None, :])
                    nc.vector.tensor_mul(w[:], g_b, a_b)
                    nc.vector.reduce_sum(s[:], w[:], axis=mybir.AxisListType.X)

                    # ns = 1 - sm; smS = sm*S; nsS = ns*S
                    nc.vector.tensor_scalar(ns[:], sm, -1.0, 1.0,
                                            mybir.AluOpType.mult,
                                            mybir.AluOpType.add)
                    nc.vector.tensor_mul(smS[:], sm, s[:])
                    nc.vector.tensor_mul(nsS[:], ns[:], s[:])

                    # outer products over the [T, T] pair grid
                    a1, b1 = broadcast_tensor_aps(smS[:, :, None], ns[:, None, :])
                    nc.vector.tensor_mul(pos[:], a1, b1)
                    a2, b2 = broadcast_tensor_aps(sm[:, :, None], nsS[:, None, :])
                    nc.vector.tensor_mul(neg[:], a2, b2)
                    a3, b3 = broadcast_tensor_aps(sm[:, :, None], ns[:, None, :])
                    nc.vector.tensor_mul(msk[:], a3, b3)

                    b0 = jb * 128
                    nc.sync.dma_start(nbr_o[k, b0:b0 + 128], g[:])
                    nc.sync.dma_start(pos_o[k, b0:b0 + 128], pos[:])
                    nc.sync.dma_start(neg_o[k, b0:b0 + 128], neg[:])
                    nc.sync.dma_start(msk_o[k, b0:b0 + 128], msk[:])

    nc.compile()
    from concourse.bass_interp import get_hw_module
    nc.m = get_hw_module(nc.m)
    names = dict(tbl=tbl.name, anch=anch.name, smi=smi.name, idxs=idxs.name,
                 nbr=nbr_o.name, pos=pos_o.name, neg=neg_o.name, msk=msk_o.name)
    return nc, names


def get_program():
    if "prog" not in _CACHE:
        _CACHE["prog"] = _build()
    return _CACHE["prog"]


def make_in_maps(semb, slabels, topk_ngh):
    """Host-side input prep: replicated table, per-core anchor rows, the
    binary same-label mask, and prepacked int16 gather indices."""
    names = get_program()[1]
    semb = np.ascontiguousarray(np.asarray(semb, dtype=np.float32))
    slabels = np.asarray(slabels)
    topk = np.asarray(topk_ngh)

    tbl_flat = semb.reshape(K * B, D)
    # same[k, b, t] = slabels[k, topk[k,b,t]] == slabels[k, b]
    gat = np.stack([slabels[k][topk[k]] for k in range(K)])       # [K, B, T]
    same = (gat == slabels[:, :, None]).astype(np.float32)         # [K, B, T]

    in_maps = []
    for c in range(NCORES):
        lo = c * BS
        anch_np = np.ascontiguousarray(
            semb.reshape(K, B, D)[:, lo:lo + BS].reshape(K, NBLK, 128, D))
        # sm layout: [partition p, (k, jb, t)] for this core's anchors
        sm_np = np.ascontiguousarray(
            same[:, lo:lo + BS].reshape(K, NBLK, 128, T)
            .transpose(2, 0, 1, 3).reshape(128, SMCOLS))
        # idx packing: per (k, jb, q) a [16, NI//16] int16 block in which
        # logical element j (= tq*128 + p) sits at [j % 16, j // 16];
        # replicated across the 8 groups of 16 partitions for the Q7 cores.
        tk = topk[:, lo:lo + BS]  # [K, BS, T]
        blocks = []
        for k in range(K):
            for jb in range(NBLK):
                tkb = tk[k, jb * 128:(jb + 1) * 128]          # [128, T]
                for q in range(NQ):
                    order = tkb[:, q * TQ:(q + 1) * TQ].T.reshape(-1)  # j = tq*128+p
                    blocks.append(order.reshape(NI // 16, 16).T)       # [16, NI//16]
        idx16 = np.concatenate(blocks, axis=1).astype(np.int16)
        idx_np = np.ascontiguousarray(np.tile(idx16, (8, 1)))
        in_maps.append({names["tbl"]: tbl_flat, names["anch"]: anch_np,
                        names["smi"]: sm_np, names["idxs"]: idx_np})
    return in_maps


def assemble(results):
    """Stitch per-core outputs back to full [K, B, ...] arrays."""
    names = get_program()[1]
    pos = np.concatenate([r[names["pos"]] for r in results], axis=1)
    neg = np.concatenate([r[names["neg"]] for r in results], axis=1)
    msk = np.concatenate([r[names["msk"]] for r in results], axis=1).astype(bool)
    nbr = np.concatenate([r[names["nbr"]] for r in results], axis=1)
    return pos, neg, msk, nbr


def kernel(semb, slabels, topk_ngh):
    from concourse import bass_utils

    nc, _ = get_program()
    in_maps = make_in_maps(semb, slabels, topk_ngh)
    res = bass_utils.run_bass_kernel_spmd(nc, in_maps, core_ids=list(range(NCORES)))
    return assemble(res.results)
